# revision 3
# baseline (speedup 1.0000x reference)
"""Trainium2 Bass kernel for nn_NetDensity (RISA net density maps).

Math (per net n with pins P_n):
  bbox: xmin/xmax/ymin/ymax over pins
  wt = RISA[min(|P_n|,46)] * net_weights[n]
  ox[i] = clip(min(xmax, b_i+2) - max(xmin, b_i), 0)   b_i = 2*i, i<256
  oy[j] likewise
  ch = wt/dy (dy>0 else 0), cv = wt/dx
  H = sum_n (ch*ox) outer oy ;  V = sum_n (cv*ox) outer oy
  out = (|H|+|V|, H, V)

Sharding: nets (and their CSR pin segments) are sharded across the 8 cores;
each core computes private 256x256 H^T/V^T partial maps which are summed on
the host (the unshard step).

Device formulation per net column j (128 nets on partitions), using the
V-shape identity  ox = relu(-Sx),  Sx = max(|b - cx| - rx, -min(dx,2))
with cx=(xmin+xmax)/2-1, rx=(dx+2)/2 (same for y):
  px = |b - cx|              [DVE ts: subtract, abs_max]
  Sx = max(px - rx, -mx)     [DVE ts: subtract, max   (both scalars per-net)]
  AH = relu(nch*Sx)          [DVE ts: mult, max]       nch = -wt/dy
  AV = relu(ncv*Sx)          [Pool/ACT alternating]    ncv = -wt/dx
  py = |b - cy|              [ACT: Abs, bias=-cy]
  Sy = max(py - ry, -my)     [Pool ts]
  B  = relu(-Sy)             [DVE ts, batched 8 columns wide at 4x mode]
  PSUM += B_half^T @ [AH|AV] -> [H^T | V^T]
"""

import numpy as np

import concourse.bass as bass
import concourse.bacc as bacc
import concourse.mybir as mybir
from concourse import tile
from concourse.bass_utils import run_bass_kernel_spmd

# Problem constants (fixed by the problem spec).
NUM_NETS = 262144
NUM_PINS = 1048576
NBX = 256
BSX = 2.0
NCORES = 8
NPC = NUM_NETS // NCORES          # nets per core: 32768
P = 128                            # SBUF partitions
NPP = NPC // P                     # nets per partition: 256
NTILES = NPP                       # one net column per tile: 256
GRP = 8                            # columns per B-batch group

_RISA_TAB = np.array(
    [1.0, 1.0, 1.0, 1.0,
     1.0828, 1.1536, 1.2206, 1.2823, 1.3385, 1.3991, 1.4493]
    + [1.6899] * 5 + [1.8924] * 5 + [2.0743] * 5 + [2.2334] * 5
    + [2.3892] * 5 + [2.5356] * 5 + [2.6625] * 5 + [2.7933],
    dtype=np.float32)

_CACHE = {}
TRACE = False          # test.py sets True to collect an NTFF profile
LAST_RESULT = None     # BassKernelResults of the most recent run


def _build(ntiles=NTILES):
    """Build + bacc-compile the per-core Bass program."""
    f32 = mybir.dt.float32
    bf16 = mybir.dt.bfloat16
    Alu = mybir.AluOpType
    Act = mybir.ActivationFunctionType

    nc = bacc.Bacc("TRN2", target_bir_lowering=False, debug=False,
                   num_devices=NCORES)
    coords_d = nc.dram_tensor("coords", [P, ntiles * 8], f32, kind="ExternalInput")
    netw_d = nc.dram_tensor("netw", [P, ntiles], f32, kind="ExternalInput")
    nrisa_d = nc.dram_tensor("nrisa", [P, ntiles], f32, kind="ExternalInput")
    brow_d = nc.dram_tensor("brow", [P, NBX], f32, kind="ExternalInput")
    out_d = nc.dram_tensor("out", [2, P, 512], f32, kind="ExternalOutput")

    with tile.TileContext(nc) as tc:
        with (
            tc.tile_pool(name="const", bufs=1) as cpool,
            tc.tile_pool(name="scal", bufs=1) as spool,
            tc.tile_pool(name="work", bufs=4) as wpool,
            tc.tile_pool(name="ahvp", bufs=2 * GRP + 2) as apool,
            tc.tile_pool(name="grp", bufs=3) as gpool,
            tc.tile_pool(name="psum", bufs=1, space="PSUM") as ppool,
        ):
            coords = cpool.tile([P, ntiles * 8], f32)
            netw = cpool.tile([P, ntiles], f32)
            nrisa = cpool.tile([P, ntiles], f32)
            brow = cpool.tile([P, NBX], f32)
            nc.sync.dma_start(out=coords[:], in_=coords_d[:, :])
            nc.sync.dma_start(out=netw[:], in_=netw_d[:, :])
            nc.sync.dma_start(out=nrisa[:], in_=nrisa_d[:, :])
            nc.sync.dma_start(out=brow[:], in_=brow_d[:, :])

            # ---- per-net scalars -------------------------------------
            # view coords as [P, net, pin, xy]
            c4 = coords[:].rearrange("p (n k t) -> p n k t", k=4, t=2)
            bbmax = spool.tile([P, ntiles * 2], f32)   # [p, net, (x,y)]
            bbmin = spool.tile([P, ntiles * 2], f32)
            ma = spool.tile([P, ntiles * 2], f32)
            mb = spool.tile([P, ntiles * 2], f32)
            mav = ma[:].rearrange("p (n t) -> p n t", t=2)
            mbv = mb[:].rearrange("p (n t) -> p n t", t=2)
            nc.vector.tensor_tensor(out=mav, in0=c4[:, :, 0, :], in1=c4[:, :, 1, :],
                                    op=Alu.max)
            nc.vector.tensor_tensor(out=mbv, in0=c4[:, :, 2, :], in1=c4[:, :, 3, :],
                                    op=Alu.max)
            nc.vector.tensor_tensor(out=bbmax[:], in0=ma[:], in1=mb[:],
                                    op=Alu.max)
            nc.vector.tensor_tensor(out=mav, in0=c4[:, :, 0, :], in1=c4[:, :, 1, :],
                                    op=Alu.min)
            nc.vector.tensor_tensor(out=mbv, in0=c4[:, :, 2, :], in1=c4[:, :, 3, :],
                                    op=Alu.min)
            nc.vector.tensor_tensor(out=bbmin[:], in0=ma[:], in1=mb[:],
                                    op=Alu.min)

            d = spool.tile([P, ntiles * 2], f32)       # (dx, dy) pairs
            nc.vector.tensor_tensor(out=d[:], in0=bbmax[:], in1=bbmin[:],
                                    op=Alu.subtract)
            dc = spool.tile([P, ntiles * 2], f32)
            nc.vector.tensor_scalar(out=dc[:], in0=d[:], scalar1=1e-12,
                                    scalar2=None, op0=Alu.max)
            rec = spool.tile([P, ntiles * 2], f32)
            nc.vector.reciprocal(out=rec[:], in_=dc[:])
            mask = spool.tile([P, ntiles * 2], f32)
            nc.vector.tensor_scalar(out=mask[:], in0=d[:], scalar1=0.0,
                                    scalar2=None, op0=Alu.is_gt)
            rm = spool.tile([P, ntiles * 2], f32)
            nc.vector.tensor_tensor(out=rm[:], in0=rec[:], in1=mask[:],
                                    op=Alu.mult)
            # negated combined weight -(risa * netw), broadcast to xy pairs
            nwt = spool.tile([P, ntiles], f32)
            nc.vector.tensor_tensor(out=nwt[:], in0=netw[:], in1=nrisa[:],
                                    op=Alu.mult)
            nwt2 = spool.tile([P, ntiles * 2], f32)
            nwt2v = nwt2[:].rearrange("p (n t) -> p n t", t=2)
            nc.vector.tensor_copy(out=nwt2v[:, :, 0], in_=nwt[:])
            nc.vector.tensor_copy(out=nwt2v[:, :, 1], in_=nwt[:])
            # nchv pairs: [.., 0] = -wt/dx = ncv ; [.., 1] = -wt/dy = nch
            nchv = spool.tile([P, ntiles * 2], f32)
            nc.vector.tensor_tensor(out=nchv[:], in0=rm[:], in1=nwt2[:],
                                    op=Alu.mult)

            # V-shape per-net scalars: center, radius, clamp.
            # cxy = (bbmax+bbmin)/2 - 1 ; ncxy = -cxy (ACT bias wants -cy)
            # rxy = d/2 + 1 ; nmxy = -min(d, 2)
            ssum = spool.tile([P, ntiles * 2], f32)
            nc.vector.tensor_tensor(out=ssum[:], in0=bbmax[:], in1=bbmin[:],
                                    op=Alu.add)
            cxy = spool.tile([P, ntiles * 2], f32)
            nc.vector.tensor_scalar(out=cxy[:], in0=ssum[:], scalar1=0.5,
                                    scalar2=-1.0, op0=Alu.mult, op1=Alu.add)
            ncxy = spool.tile([P, ntiles * 2], f32)
            nc.vector.tensor_scalar(out=ncxy[:], in0=cxy[:], scalar1=-1.0,
                                    scalar2=None, op0=Alu.mult)
            rxy = spool.tile([P, ntiles * 2], f32)
            nc.vector.tensor_scalar(out=rxy[:], in0=d[:], scalar1=0.5,
                                    scalar2=1.0, op0=Alu.mult, op1=Alu.add)
            nmxy = spool.tile([P, ntiles * 2], f32)
            nc.vector.tensor_scalar(out=nmxy[:], in0=d[:], scalar1=2.0,
                                    scalar2=-1.0, op0=Alu.min, op1=Alu.mult)

            ps0 = ppool.tile([P, 512], f32)
            ps1 = ppool.tile([P, 512], f32)

            # ---- main loop over net-column groups --------------------
            for g in range(ntiles // GRP):
                sy_all = gpool.tile([P, GRP * NBX], bf16, tag="syall")
                ahv_tiles = []
                for k in range(GRP):
                    j = g * GRP + k
                    ncx_j = ncxy[:, 2 * j:2 * j + 1]
                    ncy_j = ncxy[:, 2 * j + 1:2 * j + 2]
                    rx_j = rxy[:, 2 * j:2 * j + 1]
                    ry_j = rxy[:, 2 * j + 1:2 * j + 2]
                    nmx_j = nmxy[:, 2 * j:2 * j + 1]
                    nmy_j = nmxy[:, 2 * j + 1:2 * j + 2]
                    ncv_j = nchv[:, 2 * j:2 * j + 1]
                    nch_j = nchv[:, 2 * j + 1:2 * j + 2]

                    px = wpool.tile([P, NBX], f32, tag="px")
                    Sx = wpool.tile([P, NBX], bf16, tag="Sx")
                    py = wpool.tile([P, NBX], f32, tag="py")
                    AHV = apool.tile([P, 512], bf16, tag="AHV")

                    # x side: px = |b - cx| (ACT Abs) ; Sx = max(px-rx, -mx)
                    nc.scalar.activation(out=px[:], in_=brow[:],
                                         func=Act.Abs, bias=ncx_j, scale=1.0)
                    nc.vector.tensor_scalar(out=Sx[:], in0=px[:],
                                            scalar1=rx_j, scalar2=nmx_j,
                                            op0=Alu.subtract, op1=Alu.max)
                    nc.vector.tensor_scalar(out=AHV[:, 0:NBX], in0=Sx[:],
                                            scalar1=nch_j, scalar2=0.0,
                                            op0=Alu.mult, op1=Alu.max)
                    nc.gpsimd.tensor_scalar(out=AHV[:, NBX:512], in0=Sx[:],
                                            scalar1=ncv_j, scalar2=0.0,
                                            op0=Alu.mult, op1=Alu.max)
                    # y side: py = |b - cy| (ACT Abs) ; Sy (DVE)
                    nc.scalar.activation(out=py[:], in_=brow[:],
                                         func=Act.Abs, bias=ncy_j, scale=1.0)
                    nc.vector.tensor_scalar(out=sy_all[:, k * NBX:(k + 1) * NBX],
                                            in0=py[:],
                                            scalar1=ry_j, scalar2=nmy_j,
                                            op0=Alu.subtract, op1=Alu.max)
                    ahv_tiles.append(AHV)

                # B = relu(-Sy) for the whole group in one 4x-mode op
                bt_all = gpool.tile([P, GRP * NBX], bf16, tag="btall")
                nc.vector.tensor_scalar(out=bt_all[:], in0=sy_all[:],
                                        scalar1=-1.0, scalar2=0.0,
                                        op0=Alu.mult, op1=Alu.max)

                for k in range(GRP):
                    j = g * GRP + k
                    st = (j == 0)
                    sp = (j == ntiles - 1)
                    nc.tensor.matmul(out=ps0[:],
                                     lhsT=bt_all[:, k * NBX:k * NBX + 128],
                                     rhs=ahv_tiles[k][:], start=st, stop=sp)
                    nc.tensor.matmul(out=ps1[:],
                                     lhsT=bt_all[:, k * NBX + 128:(k + 1) * NBX],
                                     rhs=ahv_tiles[k][:], start=st, stop=sp)

            # ---- write out -------------------------------------------
            o0 = cpool.tile([P, 512], f32, tag="o0")
            o1 = cpool.tile([P, 512], f32, tag="o1")
            nc.vector.tensor_copy(out=o0[:], in_=ps0[:])
            nc.vector.tensor_copy(out=o1[:], in_=ps1[:])
            nc.sync.dma_start(out=out_d[0, :, :], in_=o0[:])
            nc.sync.dma_start(out=out_d[1, :, :], in_=o1[:])

    nc.compile()
    return nc


def _shard_inputs(pin_pos, netpin_start, flat_netpin, net_weights, ntiles=NTILES):
    """Host-side sharding: nets (and their CSR pin segments) across 8 cores."""
    nets = P * ntiles
    xy = np.asarray(pin_pos, dtype=np.float32).reshape(-1, 2)
    nps = np.asarray(netpin_start, dtype=np.int64)
    fnp = np.asarray(flat_netpin, dtype=np.int64)
    nw = np.asarray(net_weights, dtype=np.float32)

    cnt_all = nps[1:] - nps[:-1]
    nrisa_all = -_RISA_TAB[np.minimum(cnt_all, len(_RISA_TAB) - 1)]

    brow = np.broadcast_to(
        (np.arange(NBX, dtype=np.float32) * BSX)[None, :], (P, NBX)).copy()

    in_maps = []
    for c in range(NCORES):
        lo = c * nets
        sel = np.arange(lo, lo + nets)
        # pad each net's pin list to 4 by repeating its first pin
        # (doesn't change the bbox)
        starts = nps[sel]
        cnts = np.maximum(cnt_all[sel], 1)
        k = np.minimum(np.arange(4)[None, :], (cnts - 1)[:, None])
        pin_ids = fnp[starts[:, None] + k]              # [nets, 4]
        coords = xy[pin_ids.reshape(-1)]                # [nets*4, 2]
        in_maps.append({
            "coords": np.ascontiguousarray(coords.reshape(P, ntiles * 8)),
            "netw": np.ascontiguousarray(nw[sel].reshape(P, ntiles)),
            "nrisa": np.ascontiguousarray(nrisa_all[sel].reshape(P, ntiles)),
            "brow": brow,
        })
    return in_maps


def kernel(pin_pos, netpin_start, flat_netpin, net_weights):
    key = NTILES
    if key not in _CACHE:
        _CACHE[key] = _build(NTILES)
    nc = _CACHE[key]

    in_maps = _shard_inputs(pin_pos, netpin_start, flat_netpin, net_weights)
    res = run_bass_kernel_spmd(nc, in_maps, core_ids=list(range(NCORES)),
                               trace=TRACE)
    global LAST_RESULT
    LAST_RESULT = res

    # Unshard: sum the per-core partial transposed maps, then transpose.
    HT = np.zeros((256, 256), dtype=np.float32)
    VT = np.zeros((256, 256), dtype=np.float32)
    for c in range(NCORES):
        o = res.results[c]["out"]          # [2, 128, 512]
        HT[0:128] += o[0, :, 0:256]
        HT[128:256] += o[1, :, 0:256]
        VT[0:128] += o[0, :, 256:512]
        VT[128:256] += o[1, :, 256:512]
    H = np.ascontiguousarray(HT.T)
    V = np.ascontiguousarray(VT.T)
    return np.abs(H) + np.abs(V), H, V


# revision 5
# speedup vs baseline: 2.7738x; 2.7738x over previous
"""Trainium2 Bass kernel for nn_NetDensity (RISA net density maps).

Math (per net n with pins P_n):
  bbox: xmin/xmax/ymin/ymax over pins
  wt = RISA[min(|P_n|,46)] * net_weights[n]
  ox[i] = clip(min(xmax, b_i+2) - max(xmin, b_i), 0)   b_i = 2*i, i<256
  oy[j] likewise
  ch = wt/dy (dy>0 else 0), cv = wt/dx
  H = sum_n (ch*ox) outer oy ;  V = sum_n (cv*ox) outer oy
  out = (|H|+|V|, H, V)

Sharding: nets (and their CSR pin segments) are sharded across the 8 cores;
each core computes private 256x256 H^T/V^T partial maps which are summed on
the host (the unshard step).

Device math per net column j (128 nets on partitions), using the V-shape
identity  ox = relu(-Sx),  Sx = max(|b-cx| - rx, -min(dx,2)),
cx=(xmin+xmax)/2-1, rx=(dx+2)/2 (same for y).  The ch scale (x16 for fp8)
is folded into the x-side abs so the relus become batched immediate ops:
  px_h = 16*ch*|b - cx|            [ACT Abs, scale=16*nch, bias=-16*nch*cx]
  u    = max(px_h - chrx, nchmx)   [DVE ts]   = 16*ch*Sx
  AH   = relu(-u)                  [DVE, batched over 8 columns] = 16*ch*ox
  AV   = relu(nrho*u)              [Pool mult + DVE batched relu] = 16*cv*ox
  py   = |b - cy|                  [ACT Abs]
  Sy   = max(py - ry, -my)         [DVE ts]
  B    = relu(-32*Sy)              [DVE, batched] = 32*oy
  PSUM += fp8 DoubleRow matmuls over column pairs -> [H^T | V^T] * 512
Host divides the gathered maps by 512.
"""

import numpy as np

import concourse.bass as bass
import concourse.bacc as bacc
import concourse.mybir as mybir
from concourse import tile
from concourse.bass_utils import run_bass_kernel_spmd

# Problem constants (fixed by the problem spec).
NUM_NETS = 262144
NUM_PINS = 1048576
NBX = 256
BSX = 2.0
NCORES = 8
NPC = NUM_NETS // NCORES          # nets per core: 32768
P = 128                            # SBUF partitions
NPP = NPC // P                     # nets per partition: 256
NTILES = NPP                       # one net column per tile: 256
GRP = 8                            # columns per batch group
SA = 16.0                          # fp8 scale on the A (x) side
SB = 32.0                          # fp8 scale on the B (y) side

_RISA_TAB = np.array(
    [1.0, 1.0, 1.0, 1.0,
     1.0828, 1.1536, 1.2206, 1.2823, 1.3385, 1.3991, 1.4493]
    + [1.6899] * 5 + [1.8924] * 5 + [2.0743] * 5 + [2.2334] * 5
    + [2.3892] * 5 + [2.5356] * 5 + [2.6625] * 5 + [2.7933],
    dtype=np.float32)

_CACHE = {}
TRACE = False          # test.py sets True to collect an NTFF profile
LAST_RESULT = None     # BassKernelResults of the most recent run


def _build(ntiles=NTILES):
    """Build + bacc-compile the per-core Bass program."""
    f32 = mybir.dt.float32
    bf16 = mybir.dt.bfloat16
    fp8 = mybir.dt.float8e4
    Alu = mybir.AluOpType
    Act = mybir.ActivationFunctionType
    DR = mybir.MatmulPerfMode.DoubleRow

    nc = bacc.Bacc("TRN2", target_bir_lowering=False, debug=False,
                   num_devices=NCORES)
    coords_d = nc.dram_tensor("coords", [P, ntiles * 8], f32, kind="ExternalInput")
    netw_d = nc.dram_tensor("netw", [P, ntiles], f32, kind="ExternalInput")
    nrisa_d = nc.dram_tensor("nrisa", [P, ntiles], f32, kind="ExternalInput")
    brow_d = nc.dram_tensor("brow", [P, NBX], f32, kind="ExternalInput")
    out_d = nc.dram_tensor("out", [2, P, 512], f32, kind="ExternalOutput")

    with tile.TileContext(nc) as tc:
        with (
            tc.tile_pool(name="const", bufs=1) as cpool,
            tc.tile_pool(name="scal", bufs=1) as spool,
            tc.tile_pool(name="work", bufs=4) as wpool,
            tc.tile_pool(name="grp", bufs=3) as gpool,
            tc.tile_pool(name="psum", bufs=1, space="PSUM") as ppool,
        ):
            coords = cpool.tile([P, ntiles * 8], f32)
            netw = cpool.tile([P, ntiles], f32)
            nrisa = cpool.tile([P, ntiles], f32)
            brow = cpool.tile([P, NBX], f32)
            nc.sync.dma_start(out=coords[:], in_=coords_d[:, :])
            nc.sync.dma_start(out=netw[:], in_=netw_d[:, :])
            nc.sync.dma_start(out=nrisa[:], in_=nrisa_d[:, :])
            nc.sync.dma_start(out=brow[:], in_=brow_d[:, :])

            # ---- per-net scalars -------------------------------------
            # view coords as [P, net, pin, xy]
            c4 = coords[:].rearrange("p (n k t) -> p n k t", k=4, t=2)
            bbmax = spool.tile([P, ntiles * 2], f32)   # [p, net, (x,y)]
            bbmin = spool.tile([P, ntiles * 2], f32)
            ma = spool.tile([P, ntiles * 2], f32)
            mb = spool.tile([P, ntiles * 2], f32)
            mav = ma[:].rearrange("p (n t) -> p n t", t=2)
            mbv = mb[:].rearrange("p (n t) -> p n t", t=2)
            nc.vector.tensor_tensor(out=mav, in0=c4[:, :, 0, :], in1=c4[:, :, 1, :],
                                    op=Alu.max)
            nc.vector.tensor_tensor(out=mbv, in0=c4[:, :, 2, :], in1=c4[:, :, 3, :],
                                    op=Alu.max)
            nc.vector.tensor_tensor(out=bbmax[:], in0=ma[:], in1=mb[:],
                                    op=Alu.max)
            nc.vector.tensor_tensor(out=mav, in0=c4[:, :, 0, :], in1=c4[:, :, 1, :],
                                    op=Alu.min)
            nc.vector.tensor_tensor(out=mbv, in0=c4[:, :, 2, :], in1=c4[:, :, 3, :],
                                    op=Alu.min)
            nc.vector.tensor_tensor(out=bbmin[:], in0=ma[:], in1=mb[:],
                                    op=Alu.min)

            d = spool.tile([P, ntiles * 2], f32)       # (dx, dy) pairs
            nc.vector.tensor_tensor(out=d[:], in0=bbmax[:], in1=bbmin[:],
                                    op=Alu.subtract)
            dc = spool.tile([P, ntiles * 2], f32)
            nc.vector.tensor_scalar(out=dc[:], in0=d[:], scalar1=1e-12,
                                    scalar2=None, op0=Alu.max)
            rec = spool.tile([P, ntiles * 2], f32)
            nc.vector.reciprocal(out=rec[:], in_=dc[:])
            mask = spool.tile([P, ntiles * 2], f32)
            nc.vector.tensor_scalar(out=mask[:], in0=d[:], scalar1=0.0,
                                    scalar2=None, op0=Alu.is_gt)
            rm = spool.tile([P, ntiles * 2], f32)
            nc.vector.tensor_tensor(out=rm[:], in0=rec[:], in1=mask[:],
                                    op=Alu.mult)
            # negated combined weight -(16 * risa * netw) (SA folded on host)
            nwt = spool.tile([P, ntiles], f32)
            nc.vector.tensor_tensor(out=nwt[:], in0=netw[:], in1=nrisa[:],
                                    op=Alu.mult)
            nwt2 = spool.tile([P, ntiles * 2], f32)
            nwt2v = nwt2[:].rearrange("p (n t) -> p n t", t=2)
            nc.vector.tensor_copy(out=nwt2v[:, :, 0], in_=nwt[:])
            nc.vector.tensor_copy(out=nwt2v[:, :, 1], in_=nwt[:])
            # nchv pairs: [.., 0] = -SA*wt/dx = ncv ; [.., 1] = -SA*wt/dy = nch
            nchv = spool.tile([P, ntiles * 2], f32)
            nc.vector.tensor_tensor(out=nchv[:], in0=rm[:], in1=nwt2[:],
                                    op=Alu.mult)

            # V-shape per-net scalars (pairs): center, radius, clamp.
            ssum = spool.tile([P, ntiles * 2], f32)
            nc.vector.tensor_tensor(out=ssum[:], in0=bbmax[:], in1=bbmin[:],
                                    op=Alu.add)
            cxy = spool.tile([P, ntiles * 2], f32)
            nc.vector.tensor_scalar(out=cxy[:], in0=ssum[:], scalar1=0.5,
                                    scalar2=-1.0, op0=Alu.mult, op1=Alu.add)
            ncxy = spool.tile([P, ntiles * 2], f32)
            nc.vector.tensor_scalar(out=ncxy[:], in0=cxy[:], scalar1=-1.0,
                                    scalar2=None, op0=Alu.mult)
            rxy = spool.tile([P, ntiles * 2], f32)
            nc.vector.tensor_scalar(out=rxy[:], in0=d[:], scalar1=0.5,
                                    scalar2=1.0, op0=Alu.mult, op1=Alu.add)
            nmxy = spool.tile([P, ntiles * 2], f32)
            nc.vector.tensor_scalar(out=nmxy[:], in0=d[:], scalar1=2.0,
                                    scalar2=-1.0, op0=Alu.min, op1=Alu.mult)

            # x-side folded scalars, [P, ntiles] each (strided pair views):
            #   abias = nch*(-cx),  chrx = -nch*rx,  nchmx = nch*mx,
            #   nrho = -dy/dx (masked)
            nch_v = nchv[:].rearrange("p (n t) -> p n t", t=2)[:, :, 1]
            ncx_v = ncxy[:].rearrange("p (n t) -> p n t", t=2)[:, :, 0]
            rx_v = rxy[:].rearrange("p (n t) -> p n t", t=2)[:, :, 0]
            nmx_v = nmxy[:].rearrange("p (n t) -> p n t", t=2)[:, :, 0]
            dy_v = d[:].rearrange("p (n t) -> p n t", t=2)[:, :, 1]
            rmx_v = rm[:].rearrange("p (n t) -> p n t", t=2)[:, :, 0]

            abias = spool.tile([P, ntiles], f32)
            nc.vector.tensor_tensor(out=abias[:], in0=nch_v, in1=ncx_v,
                                    op=Alu.mult)
            chrx_t = spool.tile([P, ntiles], f32)      # nch*rx (<=0)
            nc.vector.tensor_tensor(out=chrx_t[:], in0=nch_v, in1=rx_v,
                                    op=Alu.mult)
            chrx = spool.tile([P, ntiles], f32)        # -nch*rx = +16*ch*rx
            nc.vector.tensor_scalar(out=chrx[:], in0=chrx_t[:], scalar1=-1.0,
                                    scalar2=None, op0=Alu.mult)
            nchmx = spool.tile([P, ntiles], f32)       # nch*mx (<=0)
            nc.vector.tensor_tensor(out=nchmx[:], in0=nch_v, in1=nmx_v,
                                    op=Alu.mult)
            nc.vector.tensor_scalar(out=nchmx[:], in0=nchmx[:], scalar1=-1.0,
                                    scalar2=None, op0=Alu.mult)
            rho_t = spool.tile([P, ntiles], f32)       # dy/dx masked
            nc.vector.tensor_tensor(out=rho_t[:], in0=dy_v, in1=rmx_v,
                                    op=Alu.mult)
            nrho = spool.tile([P, ntiles], f32)        # -dy/dx masked
            nc.vector.tensor_scalar(out=nrho[:], in0=rho_t[:], scalar1=-1.0,
                                    scalar2=None, op0=Alu.mult)

            ps0 = ppool.tile([P, 512], f32)
            ps1 = ppool.tile([P, 512], f32)

            # ---- main loop over net-column groups --------------------
            ngrp = ntiles // GRP
            for g in range(ngrp):
                u_all = gpool.tile([P, GRP * NBX], bf16, tag="uall")
                sy_all = gpool.tile([P, GRP * NBX], bf16, tag="syall")
                vv_all = gpool.tile([P, (GRP // 2) * NBX], bf16, tag="vvall")
                ahv = gpool.tile([P, GRP, 512], fp8, tag="ahv")
                bt_all = gpool.tile([P, GRP * NBX], fp8, tag="btall")

                for k in range(GRP):
                    j = g * GRP + k
                    nch_j = nchv[:, 2 * j + 1:2 * j + 2]
                    abias_j = abias[:, j:j + 1]
                    chrx_j = chrx[:, j:j + 1]
                    nchmx_j = nchmx[:, j:j + 1]
                    nrho_j = nrho[:, j:j + 1]
                    ncy_j = ncxy[:, 2 * j + 1:2 * j + 2]
                    ry_j = rxy[:, 2 * j + 1:2 * j + 2]
                    nmy_j = nmxy[:, 2 * j + 1:2 * j + 2]

                    px_h = wpool.tile([P, NBX], f32, tag="pxh")
                    py = wpool.tile([P, NBX], f32, tag="py")
                    u_slot = u_all[:, k * NBX:(k + 1) * NBX]
                    sy_slot = sy_all[:, k * NBX:(k + 1) * NBX]

                    # x side: px_h = |16*nch*b + abias| = 16*ch*|b-cx|
                    nc.scalar.activation(out=px_h[:], in_=brow[:],
                                         func=Act.Abs, bias=abias_j,
                                         scale=nch_j)
                    nc.vector.tensor_scalar(out=u_slot, in0=px_h[:],
                                            scalar1=chrx_j, scalar2=nchmx_j,
                                            op0=Alu.subtract, op1=Alu.max)
                    # y side: py = |b - cy| ; Sy = max(py-ry, -my)
                    nc.scalar.activation(out=py[:], in_=brow[:],
                                         func=Act.Abs, bias=ncy_j, scale=1.0)
                    nc.vector.tensor_scalar(out=sy_slot, in0=py[:],
                                            scalar1=ry_j, scalar2=nmy_j,
                                            op0=Alu.subtract, op1=Alu.max)
                    # AV pre-product vv = nrho*u (relu comes batched):
                    # A/B test pool-tt-broadcast (even k) vs DVE ts (odd k).
                    if k % 2 == 0:
                        nc.gpsimd.tensor_tensor(
                            out=vv_all[:, (k // 2) * NBX:(k // 2 + 1) * NBX],
                            in0=u_slot,
                            in1=nrho_j.broadcast_to((P, NBX)),
                            op=Alu.mult)
                    else:
                        nc.vector.tensor_scalar(out=ahv[:, k, NBX:512],
                                                in0=u_slot,
                                                scalar1=nrho_j, scalar2=0.0,
                                                op0=Alu.mult, op1=Alu.max)

                # Batched relus (immediate scalars, wide):
                # AH = relu(-u) ; B = relu(-32*Sy) ; AV(even) = relu(vv)
                ah_view = ahv[:, :, 0:NBX]
                nc.vector.tensor_scalar(
                    out=ah_view,
                    in0=u_all[:].rearrange("p (k x) -> p k x", k=GRP),
                    scalar1=-1.0, scalar2=0.0, op0=Alu.mult, op1=Alu.max)
                nc.vector.tensor_scalar(out=bt_all[:], in0=sy_all[:],
                                        scalar1=-SB, scalar2=0.0,
                                        op0=Alu.mult, op1=Alu.max)
                av_even = ahv[:, 0:GRP:2, NBX:512]
                nc.vector.tensor_scalar(
                    out=av_even,
                    in0=vv_all[:].rearrange("p (k x) -> p k x", k=GRP // 2),
                    scalar1=0.0, scalar2=None, op0=Alu.max)

                # fp8 DoubleRow matmuls over column pairs.
                for k2 in range(0, GRP, 2):
                    j0 = g * GRP + k2
                    st = (j0 == 0)
                    sp = (j0 == ntiles - 2)
                    lhsT = bt_all[:, k2 * NBX:(k2 + 2) * NBX].rearrange(
                        "p (ko m) -> p ko m", ko=2)
                    rhs = ahv[:, k2:k2 + 2, :]
                    nc.tensor.matmul(out=ps0[:], lhsT=lhsT[:, :, 0:128],
                                     rhs=rhs, start=st, stop=sp,
                                     perf_mode=DR)
                    nc.tensor.matmul(out=ps1[:], lhsT=lhsT[:, :, 128:256],
                                     rhs=rhs, start=st, stop=sp,
                                     perf_mode=DR)

            # ---- write out -------------------------------------------
            o0 = cpool.tile([P, 512], f32, tag="o0")
            o1 = cpool.tile([P, 512], f32, tag="o1")
            nc.vector.tensor_copy(out=o0[:], in_=ps0[:])
            nc.vector.tensor_copy(out=o1[:], in_=ps1[:])
            nc.sync.dma_start(out=out_d[0, :, :], in_=o0[:])
            nc.sync.dma_start(out=out_d[1, :, :], in_=o1[:])

    nc.compile()
    return nc


def _shard_inputs(pin_pos, netpin_start, flat_netpin, net_weights, ntiles=NTILES):
    """Host-side sharding: nets (and their CSR pin segments) across 8 cores."""
    nets = P * ntiles
    xy = np.asarray(pin_pos, dtype=np.float32).reshape(-1, 2)
    nps = np.asarray(netpin_start, dtype=np.int64)
    fnp = np.asarray(flat_netpin, dtype=np.int64)
    nw = np.asarray(net_weights, dtype=np.float32)

    cnt_all = nps[1:] - nps[:-1]
    # SA folded here so the device-side weight is -SA*risa*netw.
    nrisa_all = -SA * _RISA_TAB[np.minimum(cnt_all, len(_RISA_TAB) - 1)]

    brow = np.broadcast_to(
        (np.arange(NBX, dtype=np.float32) * BSX)[None, :], (P, NBX)).copy()

    in_maps = []
    for c in range(NCORES):
        lo = c * nets
        sel = np.arange(lo, lo + nets)
        # pad each net's pin list to 4 by repeating its first pin
        # (doesn't change the bbox)
        starts = nps[sel]
        cnts = np.maximum(cnt_all[sel], 1)
        k = np.minimum(np.arange(4)[None, :], (cnts - 1)[:, None])
        pin_ids = fnp[starts[:, None] + k]              # [nets, 4]
        coords = xy[pin_ids.reshape(-1)]                # [nets*4, 2]
        in_maps.append({
            "coords": np.ascontiguousarray(coords.reshape(P, ntiles * 8)),
            "netw": np.ascontiguousarray(nw[sel].reshape(P, ntiles)),
            "nrisa": np.ascontiguousarray(nrisa_all[sel].reshape(P, ntiles)),
            "brow": brow,
        })
    return in_maps


def kernel(pin_pos, netpin_start, flat_netpin, net_weights):
    key = NTILES
    if key not in _CACHE:
        _CACHE[key] = _build(NTILES)
    nc = _CACHE[key]

    in_maps = _shard_inputs(pin_pos, netpin_start, flat_netpin, net_weights)
    res = run_bass_kernel_spmd(nc, in_maps, core_ids=list(range(NCORES)),
                               trace=TRACE)
    global LAST_RESULT
    LAST_RESULT = res

    # Unshard: sum the per-core partial transposed maps, then transpose.
    HT = np.zeros((256, 256), dtype=np.float32)
    VT = np.zeros((256, 256), dtype=np.float32)
    for c in range(NCORES):
        o = res.results[c]["out"]          # [2, 128, 512]
        HT[0:128] += o[0, :, 0:256]
        HT[128:256] += o[1, :, 0:256]
        VT[0:128] += o[0, :, 256:512]
        VT[128:256] += o[1, :, 256:512]
    s = 1.0 / (SA * SB)
    H = np.ascontiguousarray(HT.T) * s
    V = np.ascontiguousarray(VT.T) * s
    return np.abs(H) + np.abs(V), H, V


# revision 6
# speedup vs baseline: 3.3045x; 1.1914x over previous
"""Trainium2 Bass kernel for nn_NetDensity (RISA net density maps).

Math (per net n with pins P_n):
  bbox: xmin/xmax/ymin/ymax over pins
  wt = RISA[min(|P_n|,46)] * net_weights[n]
  ox[i] = clip(min(xmax, b_i+2) - max(xmin, b_i), 0)   b_i = 2*i, i<256
  oy[j] likewise
  ch = wt/dy (dy>0 else 0), cv = wt/dx
  H = sum_n (ch*ox) outer oy ;  V = sum_n (cv*ox) outer oy
  out = (|H|+|V|, H, V)

Sharding: nets (and their CSR pin segments) are sharded across the 8 cores;
each core computes private 256x256 H^T/V^T partial maps which are summed on
the host (the unshard step).

Device formulation per 128-net column (nets on the K/partition axis):
  T1 = max(b - xmax, -2)            (= -min(xmax-b, 2))
  t2 = relu(xmin - b)               [ACT]
  Sx = T1 + t2                      (= -(ox before outer relu))
  A_H = relu(nch * Sx)   nch = -wt/dy    [DVE]
  A_V = relu(ncv * Sx)                   [ACT/DVE rotating]
  B   = relu(-Sy)                   (= oy)  [DVE, batched 8 columns wide]
  PSUM += B_chunk^T @ [A_H | A_V]   -> [H^T | V^T]

vs the original baseline: Sx|Sy go into a [P, GRP, 512] group tile so the
B relu runs once per 8 columns as a wide strided op (instead of 256 ACT
ops), and A_V moves to the Scalar engine for 3 of 4 columns to balance
DVE vs ACT load.
"""

import numpy as np

import concourse.bass as bass
import concourse.bacc as bacc
import concourse.mybir as mybir
from concourse import tile
from concourse.bass_utils import run_bass_kernel_spmd

# Problem constants (fixed by the problem spec).
NUM_NETS = 262144
NUM_PINS = 1048576
NBX = 256
BSX = 2.0
NCORES = 8
NPC = NUM_NETS // NCORES          # nets per core: 32768
P = 128                            # SBUF partitions
NPP = NPC // P                     # nets per partition: 256
NTILES = NPP                       # one net column per tile: 256
GRP = 8                            # columns per B-batch group

_RISA_TAB = np.array(
    [1.0, 1.0, 1.0, 1.0,
     1.0828, 1.1536, 1.2206, 1.2823, 1.3385, 1.3991, 1.4493]
    + [1.6899] * 5 + [1.8924] * 5 + [2.0743] * 5 + [2.2334] * 5
    + [2.3892] * 5 + [2.5356] * 5 + [2.6625] * 5 + [2.7933],
    dtype=np.float32)

_CACHE = {}
TRACE = False          # test.py sets True to collect an NTFF profile
LAST_RESULT = None     # BassKernelResults of the most recent run


def _build(ntiles=NTILES):
    """Build + bacc-compile the per-core Bass program."""
    f32 = mybir.dt.float32
    bf16 = mybir.dt.bfloat16
    Alu = mybir.AluOpType
    Act = mybir.ActivationFunctionType

    nc = bacc.Bacc("TRN2", target_bir_lowering=False, debug=False,
                   num_devices=NCORES)
    coords_d = nc.dram_tensor("coords", [P, ntiles * 8], f32, kind="ExternalInput")
    netw_d = nc.dram_tensor("netw", [P, ntiles], f32, kind="ExternalInput")
    nrisa_d = nc.dram_tensor("nrisa", [P, ntiles], f32, kind="ExternalInput")
    brow_d = nc.dram_tensor("brow", [P, NBX], f32, kind="ExternalInput")
    out_d = nc.dram_tensor("out", [2, P, 512], f32, kind="ExternalOutput")

    with tile.TileContext(nc) as tc:
        with (
            tc.tile_pool(name="const", bufs=1) as cpool,
            tc.tile_pool(name="scal", bufs=1) as spool,
            tc.tile_pool(name="work", bufs=4) as wpool,
            tc.tile_pool(name="ahvp", bufs=2 * GRP + 2) as apool,
            tc.tile_pool(name="grp", bufs=3) as gpool,
            tc.tile_pool(name="psum", bufs=1, space="PSUM") as ppool,
        ):
            coords = cpool.tile([P, ntiles * 8], f32)
            netw = cpool.tile([P, ntiles], f32)
            nrisa = cpool.tile([P, ntiles], f32)
            brow = cpool.tile([P, NBX], f32)
            browb = cpool.tile([P, NBX], bf16)
            nc.sync.dma_start(out=coords[:], in_=coords_d[:, :])
            nc.sync.dma_start(out=netw[:], in_=netw_d[:, :])
            nc.sync.dma_start(out=nrisa[:], in_=nrisa_d[:, :])
            nc.sync.dma_start(out=brow[:], in_=brow_d[:, :])
            nc.vector.tensor_copy(out=browb[:], in_=brow[:])

            # ---- per-net scalars -------------------------------------
            # view coords as [P, net, pin, xy]
            c4 = coords[:].rearrange("p (n k t) -> p n k t", k=4, t=2)
            bbmax = spool.tile([P, ntiles * 2], f32)   # [p, net, (x,y)]
            bbmin = spool.tile([P, ntiles * 2], f32)
            ma = spool.tile([P, ntiles * 2], f32)
            mb = spool.tile([P, ntiles * 2], f32)
            mav = ma[:].rearrange("p (n t) -> p n t", t=2)
            mbv = mb[:].rearrange("p (n t) -> p n t", t=2)
            nc.vector.tensor_tensor(out=mav, in0=c4[:, :, 0, :], in1=c4[:, :, 1, :],
                                    op=Alu.max)
            nc.vector.tensor_tensor(out=mbv, in0=c4[:, :, 2, :], in1=c4[:, :, 3, :],
                                    op=Alu.max)
            nc.vector.tensor_tensor(out=bbmax[:], in0=ma[:], in1=mb[:],
                                    op=Alu.max)
            nc.vector.tensor_tensor(out=mav, in0=c4[:, :, 0, :], in1=c4[:, :, 1, :],
                                    op=Alu.min)
            nc.vector.tensor_tensor(out=mbv, in0=c4[:, :, 2, :], in1=c4[:, :, 3, :],
                                    op=Alu.min)
            nc.vector.tensor_tensor(out=bbmin[:], in0=ma[:], in1=mb[:],
                                    op=Alu.min)

            d = spool.tile([P, ntiles * 2], f32)       # (dx, dy) pairs
            nc.vector.tensor_tensor(out=d[:], in0=bbmax[:], in1=bbmin[:],
                                    op=Alu.subtract)
            dc = spool.tile([P, ntiles * 2], f32)
            nc.vector.tensor_scalar(out=dc[:], in0=d[:], scalar1=1e-12,
                                    scalar2=None, op0=Alu.max)
            rec = spool.tile([P, ntiles * 2], f32)
            nc.vector.reciprocal(out=rec[:], in_=dc[:])
            mask = spool.tile([P, ntiles * 2], f32)
            nc.vector.tensor_scalar(out=mask[:], in0=d[:], scalar1=0.0,
                                    scalar2=None, op0=Alu.is_gt)
            rm = spool.tile([P, ntiles * 2], f32)
            nc.vector.tensor_tensor(out=rm[:], in0=rec[:], in1=mask[:],
                                    op=Alu.mult)
            # negated combined weight -(risa * netw), broadcast to xy pairs
            nwt = spool.tile([P, ntiles], f32)
            nc.vector.tensor_tensor(out=nwt[:], in0=netw[:], in1=nrisa[:],
                                    op=Alu.mult)
            nwt2 = spool.tile([P, ntiles * 2], f32)
            nwt2v = nwt2[:].rearrange("p (n t) -> p n t", t=2)
            nc.vector.tensor_copy(out=nwt2v[:, :, 0], in_=nwt[:])
            nc.vector.tensor_copy(out=nwt2v[:, :, 1], in_=nwt[:])
            # nchv pairs: [.., 0] = -wt/dx = ncv ; [.., 1] = -wt/dy = nch
            nchv = spool.tile([P, ntiles * 2], f32)
            nc.vector.tensor_tensor(out=nchv[:], in0=rm[:], in1=nwt2[:],
                                    op=Alu.mult)

            ps0 = ppool.tile([P, 512], f32)
            ps1 = ppool.tile([P, 512], f32)

            # ---- main loop over net-column groups --------------------
            ngrp = ntiles // GRP
            for g in range(ngrp):
                sxy_all = gpool.tile([P, GRP, 512], bf16, tag="sxy")
                ahv_tiles = []
                for k in range(GRP):
                    j = g * GRP + k
                    xmax_j = bbmax[:, 2 * j:2 * j + 1]
                    ymax_j = bbmax[:, 2 * j + 1:2 * j + 2]
                    xmin_j = bbmin[:, 2 * j:2 * j + 1]
                    ymin_j = bbmin[:, 2 * j + 1:2 * j + 2]
                    ncv_j = nchv[:, 2 * j:2 * j + 1]
                    nch_j = nchv[:, 2 * j + 1:2 * j + 2]

                    TU = wpool.tile([P, 512], bf16, tag="TU")
                    tu2 = wpool.tile([P, 512], bf16, tag="tu2")
                    AHV = apool.tile([P, 512], bf16, tag="AHV")
                    sxy = sxy_all[:, k, :]
                    sx = sxy_all[:, k, 0:NBX]

                    # T1 = max(b - xmax, -2) ; U1 = max(b - ymax, -2)  [DVE]
                    nc.vector.tensor_scalar(out=TU[:, 0:NBX], in0=browb[:],
                                            scalar1=xmax_j, scalar2=-2.0,
                                            op0=Alu.subtract, op1=Alu.max)
                    nc.vector.tensor_scalar(out=TU[:, NBX:512], in0=browb[:],
                                            scalar1=ymax_j, scalar2=-2.0,
                                            op0=Alu.subtract, op1=Alu.max)
                    # t2 = relu(xmin - b) ; u2 = relu(ymin - b)   [ACT]
                    nc.scalar.activation(out=tu2[:, 0:NBX], in_=browb[:],
                                         func=Act.Relu, bias=xmin_j, scale=-1.0)
                    nc.scalar.activation(out=tu2[:, NBX:512], in_=browb[:],
                                         func=Act.Relu, bias=ymin_j, scale=-1.0)
                    # Sx|Sy = TU + tu2 (one 512-wide op)   [DVE]
                    nc.vector.tensor_tensor(out=sxy, in0=TU[:], in1=tu2[:],
                                            op=Alu.add)
                    # A_H = relu(nch * Sx)   [DVE]
                    nc.vector.tensor_scalar(out=AHV[:, 0:NBX], in0=sx,
                                            scalar1=nch_j, scalar2=0.0,
                                            op0=Alu.mult, op1=Alu.max)
                    # A_V = relu(ncv * Sx)   [ACT 3/4, DVE 1/4]
                    if j % 4 != 3:
                        nc.scalar.activation(out=AHV[:, NBX:512], in_=sx,
                                             func=Act.Relu, scale=ncv_j)
                    else:
                        nc.vector.tensor_scalar(out=AHV[:, NBX:512], in0=sx,
                                                scalar1=ncv_j, scalar2=0.0,
                                                op0=Alu.mult, op1=Alu.max)
                    ahv_tiles.append(AHV)

                # B = relu(-Sy) for the whole group in one wide op  [DVE]
                bt_all = gpool.tile([P, GRP * NBX], bf16, tag="btall")
                nc.vector.tensor_scalar(
                    out=bt_all[:].rearrange("p (k x) -> p k x", k=GRP),
                    in0=sxy_all[:, :, NBX:512],
                    scalar1=-1.0, scalar2=0.0, op0=Alu.mult, op1=Alu.max)

                for k in range(GRP):
                    j = g * GRP + k
                    st = (j == 0)
                    sp = (j == ntiles - 1)
                    nc.tensor.matmul(out=ps0[:],
                                     lhsT=bt_all[:, k * NBX:k * NBX + 128],
                                     rhs=ahv_tiles[k][:], start=st, stop=sp)
                    nc.tensor.matmul(out=ps1[:],
                                     lhsT=bt_all[:, k * NBX + 128:(k + 1) * NBX],
                                     rhs=ahv_tiles[k][:], start=st, stop=sp)

            # ---- write out -------------------------------------------
            o0 = cpool.tile([P, 512], f32, tag="o0")
            o1 = cpool.tile([P, 512], f32, tag="o1")
            nc.vector.tensor_copy(out=o0[:], in_=ps0[:])
            nc.vector.tensor_copy(out=o1[:], in_=ps1[:])
            nc.sync.dma_start(out=out_d[0, :, :], in_=o0[:])
            nc.sync.dma_start(out=out_d[1, :, :], in_=o1[:])

    nc.compile()
    return nc


def _shard_inputs(pin_pos, netpin_start, flat_netpin, net_weights, ntiles=NTILES):
    """Host-side sharding: nets (and their CSR pin segments) across 8 cores."""
    nets = P * ntiles
    xy = np.asarray(pin_pos, dtype=np.float32).reshape(-1, 2)
    nps = np.asarray(netpin_start, dtype=np.int64)
    fnp = np.asarray(flat_netpin, dtype=np.int64)
    nw = np.asarray(net_weights, dtype=np.float32)

    cnt_all = nps[1:] - nps[:-1]
    nrisa_all = -_RISA_TAB[np.minimum(cnt_all, len(_RISA_TAB) - 1)]

    brow = np.broadcast_to(
        (np.arange(NBX, dtype=np.float32) * BSX)[None, :], (P, NBX)).copy()

    in_maps = []
    for c in range(NCORES):
        lo = c * nets
        sel = np.arange(lo, lo + nets)
        # pad each net's pin list to 4 by repeating its first pin
        # (doesn't change the bbox)
        starts = nps[sel]
        cnts = np.maximum(cnt_all[sel], 1)
        k = np.minimum(np.arange(4)[None, :], (cnts - 1)[:, None])
        pin_ids = fnp[starts[:, None] + k]              # [nets, 4]
        coords = xy[pin_ids.reshape(-1)]                # [nets*4, 2]
        in_maps.append({
            "coords": np.ascontiguousarray(coords.reshape(P, ntiles * 8)),
            "netw": np.ascontiguousarray(nw[sel].reshape(P, ntiles)),
            "nrisa": np.ascontiguousarray(nrisa_all[sel].reshape(P, ntiles)),
            "brow": brow,
        })
    return in_maps


def kernel(pin_pos, netpin_start, flat_netpin, net_weights):
    key = NTILES
    if key not in _CACHE:
        _CACHE[key] = _build(NTILES)
    nc = _CACHE[key]

    in_maps = _shard_inputs(pin_pos, netpin_start, flat_netpin, net_weights)
    res = run_bass_kernel_spmd(nc, in_maps, core_ids=list(range(NCORES)),
                               trace=TRACE)
    global LAST_RESULT
    LAST_RESULT = res

    # Unshard: sum the per-core partial transposed maps, then transpose.
    HT = np.zeros((256, 256), dtype=np.float32)
    VT = np.zeros((256, 256), dtype=np.float32)
    for c in range(NCORES):
        o = res.results[c]["out"]          # [2, 128, 512]
        HT[0:128] += o[0, :, 0:256]
        HT[128:256] += o[1, :, 0:256]
        VT[0:128] += o[0, :, 256:512]
        VT[128:256] += o[1, :, 256:512]
    H = np.ascontiguousarray(HT.T)
    V = np.ascontiguousarray(VT.T)
    return np.abs(H) + np.abs(V), H, V
